# revision 17
# baseline (speedup 1.0000x reference)
"""Trainium2 Bass kernel for nn_DynamicFeedForward (embedding-gather dot products).

Reference computation:
    part_weight = weight[mask]            # [b, s, 32, 512] gather
    out = einsum('bsh,bsmh->bsm', x, part_weight) + bias[mask]
    out = relu(out)

Strategy (data-parallel over tokens, 8 cores):
  - 8192 tokens sharded 1024/core, processed in groups of 128 (one token per
    SBUF partition).
  - The weight table is shipped as fp16 [50000, 512] (1024B rows — 2.25x less
    gather traffic than an fp32+bias augmented 2304B row). The rel-err
    budget (2e-2) dwarfs fp16 quantization (~3e-4 on the dots).
  - Weight rows are fetched with the GPSIMD dma_gather custom DMA, one
    1024-index gather per 8-candidate chunk, prefetched 2 chunks ahead so
    Pool-engine descriptor generation never gates the DMA stream.
  - int16 index range trick: the gather base points at row 32768 and the
    host supplies int16(idx - 32768); the Q7's sign-extended address math
    then reaches rows 0..49999 while staying inside the table.
    HW constraints (measured): num_idxs % 128 == 0 and num_idxs <= 1024;
    TRAILING sign-negative indices are dropped as padding (mid-list ones
    gather normally), so the host permutes token-127's candidates to end
    every gather list with a non-negative index, with an exact host-side
    fixup for any slot that would still drop.
  - Gather list order puts gathered row i at partition i%128 = token, free
    block i//128 = candidate.
  - Compute, per 8-candidate chunk (split NV/NA across engines to keep
    every engine under the DMA roofline): one batched DVE multiply for all
    8 candidates (all-fp16 packed operands hit the DVE 2x perf mode), then
    NV reduces on DVE (tensor_reduce) and NA reduces on the scalar engine
    (Copy-activation accum_out), plus one small DVE add for the biases.
    (tensor_tensor_reduce would fuse mult+reduce but hard-crashes the HW.)
  - Relu on the scalar engine, per-group store.
  - The kernel is DMA-bound: ~33.5 MB of gathered rows per core.
"""

import numpy as np

N_CORES = 8
TOKENS = 4 * 2048
HIDDEN = 512
M = 32
VOCAB = 50000
P = 128
TOK_PER_CORE = TOKENS // N_CORES          # 1024
GROUPS = TOK_PER_CORE // P                # 8
M_TILE = 8                                # candidates per dma_gather chunk
NCHUNK = M // M_TILE                      # 4
ROW = 512                                 # fp16 row elems (1024B, %256==0)
BASE = 32768                              # gather base row (int16 centering)
NIDX = P * M_TILE                         # 1024 indices per gather (%128, <=1024)

NV = 3                                    # DVE-reduced candidates per chunk
NA = 5                                    # ACT-reduced candidates per chunk
QUEUES = 4                                # parallel SWDGE gather queues: the
                                          # Q7 desc-gen is the serial wall
                                          # (~7ns/row on one queue); 4 queues
                                          # run desc-gen/trigger in parallel
PREFETCH = 2                              # gather chunks in flight ahead

_cached = None


def _build_program(repeats=1, nv=NV, na=NA, compute=True, queues=QUEUES,
                   prefetch=PREFETCH):
    import concourse.bacc as bacc
    import concourse.mybir as mybir
    import concourse.tile as tile

    assert nv + na == M_TILE

    f32 = mybir.dt.float32
    f16 = mybir.dt.float16
    i16 = mybir.dt.int16

    nc = bacc.Bacc(
        "TRN2",
        target_bir_lowering=False,
        debug=False,
        num_devices=N_CORES,
        num_swdge_queues=queues,
    )

    x_d = nc.dram_tensor("x", [P, GROUPS * HIDDEN], f16, kind="ExternalInput")
    idx_d = nc.dram_tensor(
        "idx", [P, GROUPS * NCHUNK * (NIDX // 16)], i16, kind="ExternalInput"
    )
    w_d = nc.dram_tensor("w", [VOCAB, ROW], f16, kind="ExternalInput")
    b_d = nc.dram_tensor("b", [P, GROUPS * M], f32, kind="ExternalInput")
    out_d = nc.dram_tensor("out", [TOK_PER_CORE, M], f32, kind="ExternalOutput")

    with tile.TileContext(nc) as tc:
        with (
            tc.tile_pool(name="wg", bufs=prefetch + 3) as wpool,
            tc.tile_pool(name="xt", bufs=1) as xpool,
            tc.tile_pool(name="idxt", bufs=1) as ipool,
            tc.tile_pool(name="bt", bufs=1) as bpool,
            tc.tile_pool(name="prod", bufs=3) as ppool,
            tc.tile_pool(name="rest", bufs=4) as rpool,
            tc.tile_pool(name="relut", bufs=4) as relupool,
            tc.tile_pool(name="dumpa", bufs=2) as dapool,
        ):
            # preload indices (512 KB), the x shard (1 MB fp16) and the
            # device-order biases (128 KB) upfront: keeps the steady-state
            # DMA stream pure gather.
            it_all = ipool.tile([P, GROUPS * NCHUNK * (NIDX // 16)], i16)
            nc.sync.dma_start(it_all[:], idx_d[:, :])
            x_all = xpool.tile([P, GROUPS * HIDDEN], f16)
            nc.sync.dma_start(x_all[:], x_d[:, :])
            b_all = bpool.tile([P, GROUPS * M], f32)
            nc.sync.dma_start(b_all[:], b_d[:, :])

            # chunk schedule: (g, h, split?) — the very last chunk of a
            # repeats=1 build is executed as two 512-row gathers to halve
            # the end-of-kernel tail.
            chunks = [
                (g, h)
                for _ in range(repeats)
                for g in range(GROUPS)
                for h in range(NCHUNK)
            ]
            K = len(chunks)

            def issue_gather(ci):
                g, h = chunks[ci]
                k = g * NCHUNK + h
                it = it_all[:, k * (NIDX // 16) : (k + 1) * (NIDX // 16)]
                split = ci == K - 1 and repeats == 1
                if split:
                    tiles = []
                    for sub in range(2):
                        it_s = it[:, sub * 32 : (sub + 1) * 32]
                        w_t = wpool.tile([P, (M_TILE // 2) * ROW], f16)
                        nc.gpsimd.dma_gather(
                            out_ap=w_t[:].rearrange("p (c e) -> p c e", e=ROW),
                            in_ap=w_d[BASE:, :],
                            idxs_ap=it_s,
                            num_idxs=NIDX // 2,
                            num_idxs_reg=NIDX // 2,
                            elem_size=ROW,
                            queue_num=(2 * ci + sub) % queues,
                        )
                        tiles.append(w_t)
                    return tiles
                w_t = wpool.tile([P, M_TILE * ROW], f16)
                nc.gpsimd.dma_gather(
                    out_ap=w_t[:].rearrange("p (c e) -> p c e", e=ROW),
                    in_ap=w_d[BASE:, :],
                    idxs_ap=it,
                    num_idxs=NIDX,
                    num_idxs_reg=NIDX,
                    elem_size=ROW,
                    queue_num=ci % queues,
                )
                return [w_t]

            def compute_chunk(w_tiles, g, h, x_t, res_t):
                # candidate c of chunk h lives at w column block c, result
                # column mm = h*M_TILE + c.  All 8 go through the batched
                # multiply; the first na reduce on ACT, the rest on DVE.
                if len(w_tiles) == 1:
                    w_views = [(w_tiles[0], 0, M_TILE)]  # (tile, c0, ncand)
                else:
                    w_views = [
                        (w_tiles[0], 0, M_TILE // 2),
                        (w_tiles[1], M_TILE // 2, M_TILE // 2),
                    ]

                # one batched multiply per gathered view (all 8 candidates)
                prods = []  # (prod_tile, mm0, ncand)
                for w_t, c0, ncand in w_views:
                    prod = ppool.tile([P, ncand * HIDDEN], f16)
                    nc.vector.tensor_tensor(
                        out=prod[:].rearrange("p (c e) -> p c e", e=HIDDEN),
                        in0=w_t[:].rearrange("p (c e) -> p c e", e=ROW),
                        in1=x_t[:, None, :].to_broadcast([P, ncand, HIDDEN]),
                        op=mybir.AluOpType.mult,
                    )
                    prods.append((prod, h * M_TILE + c0, ncand))

                # reduces over the product slices: ACT first (it lags the
                # DVE multiply anyway), DVE takes the rest
                seq = 0  # 0..M_TILE-1 across the chunk's candidates
                for prod, mm0, n in prods:
                    for j in range(n):
                        mm = mm0 + j
                        pslice = prod[:, j * HIDDEN : (j + 1) * HIDDEN]
                        if seq < na:
                            dump = dapool.tile([P, HIDDEN], f16)
                            nc.scalar.activation(
                                out=dump[:],
                                in_=pslice,
                                func=mybir.ActivationFunctionType.Copy,
                                accum_out=res_t[:, mm : mm + 1],
                            )
                        else:
                            nc.vector.tensor_reduce(
                                out=res_t[:, mm : mm + 1],
                                in_=pslice,
                                axis=mybir.AxisListType.X,
                                op=mybir.AluOpType.add,
                            )
                        seq += 1

                # biases for the whole chunk in one small DVE add
                cols = slice(h * M_TILE, (h + 1) * M_TILE)
                gcols = slice(g * M + h * M_TILE, g * M + (h + 1) * M_TILE)
                nc.vector.tensor_tensor(
                    out=res_t[:, cols],
                    in0=res_t[:, cols],
                    in1=b_all[:, gcols],
                    op=mybir.AluOpType.add,
                )

            pending = {}
            for ci in range(min(prefetch, K)):
                pending[ci] = issue_gather(ci)

            res_t = None
            for ci in range(K):
                g, h = chunks[ci]
                if ci + prefetch < K:
                    pending[ci + prefetch] = issue_gather(ci + prefetch)
                w_tiles = pending.pop(ci)

                if not compute:  # debug: pure gather stream timing
                    continue
                if h == 0:
                    res_t = rpool.tile([P, M], f32)
                x_t = x_all[:, g * HIDDEN : (g + 1) * HIDDEN]
                compute_chunk(w_tiles, g, h, x_t, res_t)

                if h == NCHUNK - 1:
                    relu_t = relupool.tile([P, M], f32)
                    nc.scalar.activation(
                        relu_t[:], res_t[:], mybir.ActivationFunctionType.Relu
                    )
                    nc.sync.dma_start(
                        out_d[g * P : (g + 1) * P, :], relu_t[:]
                    )

    nc.compile()
    return nc


def _get_program():
    global _cached
    if _cached is None:
        _cached = _build_program()
    return _cached


def _plan_core(idx):
    """Plan one core's gather lists.

    idx: [TOK_PER_CORE, M] int64/int32 original indices.

    Returns (packed, cand_order, drops):
      packed: [GROUPS*NCHUNK, P, NIDX//16] int16 device index input
      cand_order: [TOK_PER_CORE, M] int; device res column k of token t holds
        candidate cand_order[t, k]
      drops: list of (t, k) device res slots that the HW will drop
        (trailing-negative padding rule) and the host must fix up
    """
    idx = idx.astype(np.int64)
    idx16 = (idx - BASE).astype(np.int16)  # [T, M]

    cand_order = np.tile(np.arange(M), (TOK_PER_CORE, 1))
    # For each group, permute the partition-127 token's candidates so each
    # chunk's final gather-list slot (token 127, block M_TILE-1) is >= 0.
    for g in range(GROUPS):
        t = g * P + (P - 1)
        high = np.flatnonzero(idx[t] >= BASE)
        low = np.flatnonzero(idx[t] < BASE)
        order = np.empty(M, np.int64)
        tail_slots = [h * M_TILE + (M_TILE - 1) for h in range(NCHUNK)]
        if g == GROUPS - 1:
            # the final chunk executes as two 512-row gathers; its first
            # half's tail slot (col M - M_TILE//2 - 1) needs a high too
            tail_slots.append(M - M_TILE // 2 - 1)
        nh = min(len(high), len(tail_slots))
        order[tail_slots[:nh]] = high[:nh]
        rest = np.concatenate([high[nh:], low])
        other_slots = [k for k in range(M) if k not in tail_slots[:nh]]
        order[other_slots] = rest
        cand_order[t] = order

    eff = np.take_along_axis(idx16, cand_order, axis=1)  # [T, M] device order

    packed = np.empty((GROUPS * NCHUNK, P, NIDX // 16), np.int16)
    drops = []
    for g in range(GROUPS):
        blk = eff[g * P : (g + 1) * P]  # [128, M]
        for h in range(NCHUNK):
            lst = blk[:, h * M_TILE : (h + 1) * M_TILE].T.reshape(NIDX).copy()
            # the device executes the final chunk as two 512-row gathers;
            # apply the force-tail + trailing-drop rules per executed list
            split = g == GROUPS - 1 and h == NCHUNK - 1
            halves = [(0, NIDX // 2), (NIDX // 2, NIDX)] if split else [(0, NIDX)]
            for lo, hi in halves:
                if lst[hi - 1] < 0:
                    # A list that ends sign-negative loses its tail (and an
                    # all-negative list hard-faults the Q7) — force a valid
                    # dummy index and let the host recompute that one slot.
                    lst[hi - 1] = 0
                    drops.append(
                        (g * P + (P - 1), h * M_TILE + (hi - 1) // P)
                    )
                sub = lst[lo:hi]
                nonneg = np.flatnonzero(sub >= 0)
                last = nonneg[-1] if len(nonneg) else -1
                for i in range(last + 1, hi - lo):
                    p, c = i % P, (lo + i) // P
                    drops.append((g * P + p, h * M_TILE + c))
            wrapped = lst.reshape(NIDX // 16, 16).T  # [16, NIDX//16]
            packed[g * NCHUNK + h] = np.tile(wrapped, (8, 1))
    return packed, cand_order, drops


def _build_in_maps(input_value, mask_tensor, weight, bias):
    """Device input maps (one per core) + per-core plans for unpacking."""
    x = np.ascontiguousarray(
        np.asarray(input_value).reshape(TOKENS, HIDDEN), dtype=np.float32
    )
    idx = np.asarray(mask_tensor).reshape(TOKENS, M)

    w16 = np.ascontiguousarray(np.asarray(weight, np.float32).astype(np.float16))
    b32 = np.asarray(bias, np.float32)

    in_maps = []
    plans = []
    for c in range(N_CORES):
        t = slice(c * TOK_PER_CORE, (c + 1) * TOK_PER_CORE)
        packed, cand_order, drops = _plan_core(idx[t])
        plans.append((cand_order, drops))
        # device layouts: idx [P, k*s] (partition-major), x [P, g*HIDDEN]
        packed_pm = np.ascontiguousarray(
            packed.transpose(1, 0, 2).reshape(P, -1)
        )
        x_pm = np.ascontiguousarray(
            x[t]
            .reshape(GROUPS, P, HIDDEN)
            .transpose(1, 0, 2)
            .reshape(P, -1)
            .astype(np.float16)
        )
        # device-order biases: b_pm[p, g*M + m] = bias[idx[g*P+p, order[m]]]
        bsel = np.take_along_axis(b32[idx[t]], cand_order, axis=1)
        b_pm = np.ascontiguousarray(
            bsel.reshape(GROUPS, P, M).transpose(1, 0, 2).reshape(P, -1)
        ).astype(np.float32)
        in_maps.append({"x": x_pm, "idx": packed_pm, "w": w16, "b": b_pm})
    return in_maps, plans


def kernel(input_value, mask_tensor, weight, bias):
    from concourse.bass_utils import run_bass_kernel_spmd

    x = np.ascontiguousarray(
        np.asarray(input_value).reshape(TOKENS, HIDDEN), dtype=np.float32
    )
    idx = np.asarray(mask_tensor).reshape(TOKENS, M)

    nc = _get_program()
    in_maps, plans = _build_in_maps(input_value, mask_tensor, weight, bias)

    res = run_bass_kernel_spmd(nc, in_maps, core_ids=list(range(N_CORES)))
    kernel._last_results = res

    outs = []
    w32 = np.asarray(weight, np.float32)
    b32 = np.asarray(bias, np.float32)
    for c in range(N_CORES):
        dev = np.array(res.results[c]["out"])  # [T, M] in device cand order
        cand_order, drops = plans[c]
        t0 = c * TOK_PER_CORE
        for t_loc, k in drops:  # exact host fixup for HW-dropped tail slots
            cand = cand_order[t_loc, k]
            v = int(idx[t0 + t_loc, cand])
            dev[t_loc, k] = max(
                float(np.dot(x[t0 + t_loc], w32[v]) + b32[v]), 0.0
            )
        out = np.empty_like(dev)
        np.put_along_axis(out, cand_order, dev, axis=1)
        outs.append(out)

    out = np.concatenate(outs, axis=0)
    return out.reshape(mask_tensor.shape).astype(np.float32)


# revision 18
# speedup vs baseline: 1.3285x; 1.3285x over previous
"""Trainium2 Bass kernel for nn_DynamicFeedForward (embedding-gather dot products).

Reference computation:
    part_weight = weight[mask]            # [b, s, 32, 512] gather
    out = einsum('bsh,bsmh->bsm', x, part_weight) + bias[mask]
    out = relu(out)

Strategy (data-parallel over tokens, 8 cores):
  - 8192 tokens sharded 1024/core, processed in groups of 128 (one token per
    SBUF partition).
  - The weight table is shipped as fp16 [50000, 512] (1024B rows — 2.25x less
    gather traffic than an fp32+bias augmented 2304B row). The rel-err
    budget (2e-2) dwarfs fp16 quantization (~3e-4 on the dots).
  - Weight rows are fetched with the GPSIMD dma_gather custom DMA, one
    1024-index gather per 8-candidate chunk, prefetched 2 chunks ahead so
    Pool-engine descriptor generation never gates the DMA stream.
  - int16 index range trick: the gather base points at row 32768 and the
    host supplies int16(idx - 32768); the Q7's sign-extended address math
    then reaches rows 0..49999 while staying inside the table.
    HW constraints (measured): num_idxs % 128 == 0 and num_idxs <= 1024;
    TRAILING sign-negative indices are dropped as padding (mid-list ones
    gather normally), so the host permutes token-127's candidates to end
    every gather list with a non-negative index, with an exact host-side
    fixup for any slot that would still drop.
  - Gather list order puts gathered row i at partition i%128 = token, free
    block i//128 = candidate.
  - Compute, per 8-candidate chunk (split NV/NA across engines to keep
    every engine under the DMA roofline): one batched DVE multiply for all
    8 candidates (all-fp16 packed operands hit the DVE 2x perf mode), then
    NV reduces on DVE (tensor_reduce) and NA reduces on the scalar engine
    (Copy-activation accum_out), plus one small DVE add for the biases.
    (tensor_tensor_reduce would fuse mult+reduce but hard-crashes the HW.)
  - Relu on the scalar engine, per-group store.
  - The kernel is DMA-bound: ~33.5 MB of gathered rows per core.
"""

import numpy as np

N_CORES = 8
TOKENS = 4 * 2048
HIDDEN = 512
M = 32
VOCAB = 50000
P = 128
TOK_PER_CORE = TOKENS // N_CORES          # 1024
GROUPS = TOK_PER_CORE // P                # 8
M_TILE = 8                                # candidates per dma_gather chunk
NCHUNK = M // M_TILE                      # 4
ROW = 512                                 # fp16 row elems (1024B, %256==0)
BASE = 32768                              # gather base row (int16 centering)
NIDX = P * M_TILE                         # 1024 indices per gather (%128, <=1024)

NV = 3                                    # DVE-reduced candidates per chunk
NA = 5                                    # ACT-reduced candidates per chunk
QUEUES = 4                                # parallel SWDGE gather queues: the
                                          # Q7 desc-gen is the serial wall
                                          # (~7ns/row on one queue); 4 queues
                                          # run desc-gen/trigger in parallel
PREFETCH = 2                              # gather chunks in flight ahead

_cached = None


def _build_program(repeats=1, nv=NV, na=NA, compute=True, queues=QUEUES,
                   prefetch=PREFETCH, pbufs=3, dump8=False):
    import concourse.bacc as bacc
    import concourse.mybir as mybir
    import concourse.tile as tile

    assert nv + na == M_TILE

    f32 = mybir.dt.float32
    f16 = mybir.dt.float16
    i16 = mybir.dt.int16

    nc = bacc.Bacc(
        "TRN2",
        target_bir_lowering=False,
        debug=False,
        num_devices=N_CORES,
        num_swdge_queues=queues,
    )

    x_d = nc.dram_tensor("x", [P, GROUPS * HIDDEN], f16, kind="ExternalInput")
    idx_d = nc.dram_tensor(
        "idx", [P, GROUPS * NCHUNK * (NIDX // 16)], i16, kind="ExternalInput"
    )
    w_d = nc.dram_tensor("w", [VOCAB, ROW], f16, kind="ExternalInput")
    b_d = nc.dram_tensor("b", [P, GROUPS * M], f32, kind="ExternalInput")
    out_d = nc.dram_tensor("out", [TOK_PER_CORE, M], f32, kind="ExternalOutput")

    with tile.TileContext(nc) as tc:
        with (
            tc.tile_pool(name="wg", bufs=prefetch + 3) as wpool,
            tc.tile_pool(name="xt", bufs=1) as xpool,
            tc.tile_pool(name="idxt", bufs=1) as ipool,
            tc.tile_pool(name="bt", bufs=1) as bpool,
            tc.tile_pool(name="prod", bufs=pbufs) as ppool,
            tc.tile_pool(name="rest", bufs=4) as rpool,
            tc.tile_pool(name="relut", bufs=4) as relupool,
            tc.tile_pool(name="dumpa", bufs=2) as dapool,
        ):
            # preload indices (512 KB), the x shard (1 MB fp16) and the
            # device-order biases (128 KB) upfront: keeps the steady-state
            # DMA stream pure gather.
            it_all = ipool.tile([P, GROUPS * NCHUNK * (NIDX // 16)], i16)
            nc.sync.dma_start(it_all[:], idx_d[:, :])
            x_all = xpool.tile([P, GROUPS * HIDDEN], f16)
            nc.sync.dma_start(x_all[:], x_d[:, :])
            b_all = bpool.tile([P, GROUPS * M], f32)
            nc.sync.dma_start(b_all[:], b_d[:, :])

            # chunk schedule: (g, h, split?) — the very last chunk of a
            # repeats=1 build is executed as two 512-row gathers to halve
            # the end-of-kernel tail.
            chunks = [
                (g, h)
                for _ in range(repeats)
                for g in range(GROUPS)
                for h in range(NCHUNK)
            ]
            K = len(chunks)

            def issue_gather(ci):
                g, h = chunks[ci]
                k = g * NCHUNK + h
                it = it_all[:, k * (NIDX // 16) : (k + 1) * (NIDX // 16)]
                split = ci == K - 1 and repeats == 1
                if split:
                    tiles = []
                    for sub in range(2):
                        it_s = it[:, sub * 32 : (sub + 1) * 32]
                        w_t = wpool.tile([P, (M_TILE // 2) * ROW], f16)
                        nc.gpsimd.dma_gather(
                            out_ap=w_t[:].rearrange("p (c e) -> p c e", e=ROW),
                            in_ap=w_d[BASE:, :],
                            idxs_ap=it_s,
                            num_idxs=NIDX // 2,
                            num_idxs_reg=NIDX // 2,
                            elem_size=ROW,
                            queue_num=(2 * ci + sub) % queues,
                        )
                        tiles.append(w_t)
                    return tiles
                w_t = wpool.tile([P, M_TILE * ROW], f16)
                nc.gpsimd.dma_gather(
                    out_ap=w_t[:].rearrange("p (c e) -> p c e", e=ROW),
                    in_ap=w_d[BASE:, :],
                    idxs_ap=it,
                    num_idxs=NIDX,
                    num_idxs_reg=NIDX,
                    elem_size=ROW,
                    queue_num=ci % queues,
                )
                return [w_t]

            def compute_chunk(w_tiles, g, h, x_t, res_t):
                # candidate c of chunk h lives at w column block c, result
                # column mm = h*M_TILE + c.  All 8 go through the batched
                # multiply; the first na reduce on ACT, the rest on DVE.
                if len(w_tiles) == 1:
                    w_views = [(w_tiles[0], 0, M_TILE)]  # (tile, c0, ncand)
                else:
                    w_views = [
                        (w_tiles[0], 0, M_TILE // 2),
                        (w_tiles[1], M_TILE // 2, M_TILE // 2),
                    ]

                # one batched multiply per gathered view (all 8 candidates)
                prods = []  # (prod_tile, mm0, ncand)
                for w_t, c0, ncand in w_views:
                    prod = ppool.tile([P, ncand * HIDDEN], f16)
                    nc.vector.tensor_tensor(
                        out=prod[:].rearrange("p (c e) -> p c e", e=HIDDEN),
                        in0=w_t[:].rearrange("p (c e) -> p c e", e=ROW),
                        in1=x_t[:, None, :].to_broadcast([P, ncand, HIDDEN]),
                        op=mybir.AluOpType.mult,
                    )
                    prods.append((prod, h * M_TILE + c0, ncand))

                # reduces over the product slices: ACT first (it lags the
                # DVE multiply anyway), DVE takes the rest
                seq = 0  # 0..M_TILE-1 across the chunk's candidates
                for prod, mm0, n in prods:
                    for j in range(n):
                        mm = mm0 + j
                        pslice = prod[:, j * HIDDEN : (j + 1) * HIDDEN]
                        if seq < na:
                            dump = dapool.tile(
                                [P, HIDDEN],
                                mybir.dt.float8e4 if dump8 else f16,
                            )
                            nc.scalar.activation(
                                out=dump[:],
                                in_=pslice,
                                func=mybir.ActivationFunctionType.Copy,
                                accum_out=res_t[:, mm : mm + 1],
                            )
                        else:
                            nc.vector.tensor_reduce(
                                out=res_t[:, mm : mm + 1],
                                in_=pslice,
                                axis=mybir.AxisListType.X,
                                op=mybir.AluOpType.add,
                            )
                        seq += 1

                # biases for the whole chunk in one small DVE add
                cols = slice(h * M_TILE, (h + 1) * M_TILE)
                gcols = slice(g * M + h * M_TILE, g * M + (h + 1) * M_TILE)
                nc.vector.tensor_tensor(
                    out=res_t[:, cols],
                    in0=res_t[:, cols],
                    in1=b_all[:, gcols],
                    op=mybir.AluOpType.add,
                )

            pending = {}
            for ci in range(min(prefetch, K)):
                pending[ci] = issue_gather(ci)

            res_t = None
            for ci in range(K):
                g, h = chunks[ci]
                if ci + prefetch < K:
                    pending[ci + prefetch] = issue_gather(ci + prefetch)
                w_tiles = pending.pop(ci)

                if not compute:  # debug: pure gather stream timing
                    continue
                if h == 0:
                    res_t = rpool.tile([P, M], f32)
                x_t = x_all[:, g * HIDDEN : (g + 1) * HIDDEN]
                compute_chunk(w_tiles, g, h, x_t, res_t)

                if h == NCHUNK - 1:
                    relu_t = relupool.tile([P, M], f32)
                    nc.scalar.activation(
                        relu_t[:], res_t[:], mybir.ActivationFunctionType.Relu
                    )
                    nc.sync.dma_start(
                        out_d[g * P : (g + 1) * P, :], relu_t[:]
                    )

    nc.compile()
    return nc


def _get_program():
    global _cached
    if _cached is None:
        _cached = _build_program()
    return _cached


def _plan_core(idx):
    """Plan one core's gather lists.

    idx: [TOK_PER_CORE, M] int64/int32 original indices.

    Returns (packed, cand_order, drops):
      packed: [GROUPS*NCHUNK, P, NIDX//16] int16 device index input
      cand_order: [TOK_PER_CORE, M] int; device res column k of token t holds
        candidate cand_order[t, k]
      drops: list of (t, k) device res slots that the HW will drop
        (trailing-negative padding rule) and the host must fix up
    """
    idx = idx.astype(np.int64)
    idx16 = (idx - BASE).astype(np.int16)  # [T, M]

    cand_order = np.tile(np.arange(M), (TOK_PER_CORE, 1))
    # For each group, permute the partition-127 token's candidates so each
    # chunk's final gather-list slot (token 127, block M_TILE-1) is >= 0.
    for g in range(GROUPS):
        t = g * P + (P - 1)
        high = np.flatnonzero(idx[t] >= BASE)
        low = np.flatnonzero(idx[t] < BASE)
        order = np.empty(M, np.int64)
        tail_slots = [h * M_TILE + (M_TILE - 1) for h in range(NCHUNK)]
        if g == GROUPS - 1:
            # the final chunk executes as two 512-row gathers; its first
            # half's tail slot (col M - M_TILE//2 - 1) needs a high too
            tail_slots.append(M - M_TILE // 2 - 1)
        nh = min(len(high), len(tail_slots))
        order[tail_slots[:nh]] = high[:nh]
        rest = np.concatenate([high[nh:], low])
        other_slots = [k for k in range(M) if k not in tail_slots[:nh]]
        order[other_slots] = rest
        cand_order[t] = order

    eff = np.take_along_axis(idx16, cand_order, axis=1)  # [T, M] device order

    packed = np.empty((GROUPS * NCHUNK, P, NIDX // 16), np.int16)
    drops = []
    for g in range(GROUPS):
        blk = eff[g * P : (g + 1) * P]  # [128, M]
        for h in range(NCHUNK):
            lst = blk[:, h * M_TILE : (h + 1) * M_TILE].T.reshape(NIDX).copy()
            # the device executes the final chunk as two 512-row gathers;
            # apply the force-tail + trailing-drop rules per executed list
            split = g == GROUPS - 1 and h == NCHUNK - 1
            halves = [(0, NIDX // 2), (NIDX // 2, NIDX)] if split else [(0, NIDX)]
            for lo, hi in halves:
                if lst[hi - 1] < 0:
                    # A list that ends sign-negative loses its tail (and an
                    # all-negative list hard-faults the Q7) — force a valid
                    # dummy index and let the host recompute that one slot.
                    lst[hi - 1] = 0
                    drops.append(
                        (g * P + (P - 1), h * M_TILE + (hi - 1) // P)
                    )
                sub = lst[lo:hi]
                nonneg = np.flatnonzero(sub >= 0)
                last = nonneg[-1] if len(nonneg) else -1
                for i in range(last + 1, hi - lo):
                    p, c = i % P, (lo + i) // P
                    drops.append((g * P + p, h * M_TILE + c))
            wrapped = lst.reshape(NIDX // 16, 16).T  # [16, NIDX//16]
            packed[g * NCHUNK + h] = np.tile(wrapped, (8, 1))
    return packed, cand_order, drops


def _build_in_maps(input_value, mask_tensor, weight, bias):
    """Device input maps (one per core) + per-core plans for unpacking."""
    x = np.ascontiguousarray(
        np.asarray(input_value).reshape(TOKENS, HIDDEN), dtype=np.float32
    )
    idx = np.asarray(mask_tensor).reshape(TOKENS, M)

    w16 = np.ascontiguousarray(np.asarray(weight, np.float32).astype(np.float16))
    b32 = np.asarray(bias, np.float32)

    in_maps = []
    plans = []
    for c in range(N_CORES):
        t = slice(c * TOK_PER_CORE, (c + 1) * TOK_PER_CORE)
        packed, cand_order, drops = _plan_core(idx[t])
        plans.append((cand_order, drops))
        # device layouts: idx [P, k*s] (partition-major), x [P, g*HIDDEN]
        packed_pm = np.ascontiguousarray(
            packed.transpose(1, 0, 2).reshape(P, -1)
        )
        x_pm = np.ascontiguousarray(
            x[t]
            .reshape(GROUPS, P, HIDDEN)
            .transpose(1, 0, 2)
            .reshape(P, -1)
            .astype(np.float16)
        )
        # device-order biases: b_pm[p, g*M + m] = bias[idx[g*P+p, order[m]]]
        bsel = np.take_along_axis(b32[idx[t]], cand_order, axis=1)
        b_pm = np.ascontiguousarray(
            bsel.reshape(GROUPS, P, M).transpose(1, 0, 2).reshape(P, -1)
        ).astype(np.float32)
        in_maps.append({"x": x_pm, "idx": packed_pm, "w": w16, "b": b_pm})
    return in_maps, plans


def kernel(input_value, mask_tensor, weight, bias):
    from concourse.bass_utils import run_bass_kernel_spmd

    x = np.ascontiguousarray(
        np.asarray(input_value).reshape(TOKENS, HIDDEN), dtype=np.float32
    )
    idx = np.asarray(mask_tensor).reshape(TOKENS, M)

    nc = _get_program()
    in_maps, plans = _build_in_maps(input_value, mask_tensor, weight, bias)

    res = run_bass_kernel_spmd(nc, in_maps, core_ids=list(range(N_CORES)))
    kernel._last_results = res

    outs = []
    w32 = np.asarray(weight, np.float32)
    b32 = np.asarray(bias, np.float32)
    for c in range(N_CORES):
        dev = np.array(res.results[c]["out"])  # [T, M] in device cand order
        cand_order, drops = plans[c]
        t0 = c * TOK_PER_CORE
        for t_loc, k in drops:  # exact host fixup for HW-dropped tail slots
            cand = cand_order[t_loc, k]
            v = int(idx[t0 + t_loc, cand])
            dev[t_loc, k] = max(
                float(np.dot(x[t0 + t_loc], w32[v]) + b32[v]), 0.0
            )
        out = np.empty_like(dev)
        np.put_along_axis(out, cand_order, dev, axis=1)
        outs.append(out)

    out = np.concatenate(outs, axis=0)
    return out.reshape(mask_tensor.shape).astype(np.float32)


# revision 19
# speedup vs baseline: 1.3554x; 1.0203x over previous
"""Trainium2 Bass kernel for nn_DynamicFeedForward (embedding-gather dot products).

Reference computation:
    part_weight = weight[mask]            # [b, s, 32, 512] gather
    out = einsum('bsh,bsmh->bsm', x, part_weight) + bias[mask]
    out = relu(out)

Strategy (data-parallel over tokens, 8 cores):
  - 8192 tokens sharded 1024/core, processed in groups of 128 (one token per
    SBUF partition).
  - The weight table is shipped as fp16 [50000, 512] (1024B rows — 2.25x less
    gather traffic than an fp32+bias augmented 2304B row). The rel-err
    budget (2e-2) dwarfs fp16 quantization (~3e-4 on the dots).
  - Weight rows are fetched with the GPSIMD dma_gather custom DMA, one
    1024-index gather per 8-candidate chunk, prefetched 2 chunks ahead so
    Pool-engine descriptor generation never gates the DMA stream.
  - int16 index range trick: the gather base points at row 32768 and the
    host supplies int16(idx - 32768); the Q7's sign-extended address math
    then reaches rows 0..49999 while staying inside the table.
    HW constraints (measured): num_idxs % 128 == 0 and num_idxs <= 1024;
    TRAILING sign-negative indices are dropped as padding (mid-list ones
    gather normally), so the host permutes token-127's candidates to end
    every gather list with a non-negative index, with an exact host-side
    fixup for any slot that would still drop.
  - Gather list order puts gathered row i at partition i%128 = token, free
    block i//128 = candidate.
  - Compute, per 8-candidate chunk (split NV/NA across engines to keep
    every engine under the DMA roofline): one batched DVE multiply for all
    8 candidates (all-fp16 packed operands hit the DVE 2x perf mode), then
    NV reduces on DVE (tensor_reduce) and NA reduces on the scalar engine
    (Copy-activation accum_out), plus one small DVE add for the biases.
    (tensor_tensor_reduce would fuse mult+reduce but hard-crashes the HW.)
  - Relu on the scalar engine, per-group store.
  - The kernel is DMA-bound: ~33.5 MB of gathered rows per core.
"""

import numpy as np

N_CORES = 8
TOKENS = 4 * 2048
HIDDEN = 512
M = 32
VOCAB = 50000
P = 128
TOK_PER_CORE = TOKENS // N_CORES          # 1024
GROUPS = TOK_PER_CORE // P                # 8
M_TILE = 8                                # candidates per dma_gather chunk
NCHUNK = M // M_TILE                      # 4
ROW = 512                                 # fp16 row elems (1024B, %256==0)
BASE = 32768                              # gather base row (int16 centering)
NIDX = P * M_TILE                         # 1024 indices per gather (%128, <=1024)

NV = 3                                    # DVE-reduced candidates per chunk
NA = 5                                    # ACT-reduced candidates per chunk
QUEUES = 4                                # parallel SWDGE gather queues: the
                                          # Q7 desc-gen is the serial wall
                                          # (~7ns/row on one queue); 4 queues
                                          # run desc-gen/trigger in parallel
PREFETCH = 2                              # gather chunks in flight ahead

_cached = None


def _build_program(repeats=1, nv=NV, na=NA, compute=True, queues=QUEUES,
                   prefetch=PREFETCH, pbufs=5, dump8=False):
    import concourse.bacc as bacc
    import concourse.mybir as mybir
    import concourse.tile as tile

    assert nv + na == M_TILE

    f32 = mybir.dt.float32
    f16 = mybir.dt.float16
    i16 = mybir.dt.int16

    nc = bacc.Bacc(
        "TRN2",
        target_bir_lowering=False,
        debug=False,
        num_devices=N_CORES,
        num_swdge_queues=queues,
    )

    x_d = nc.dram_tensor("x", [P, GROUPS * HIDDEN], f16, kind="ExternalInput")
    idx_d = nc.dram_tensor(
        "idx", [P, GROUPS * NCHUNK * (NIDX // 16)], i16, kind="ExternalInput"
    )
    w_d = nc.dram_tensor("w", [VOCAB, ROW], f16, kind="ExternalInput")
    b_d = nc.dram_tensor("b", [P, GROUPS * M], f32, kind="ExternalInput")
    out_d = nc.dram_tensor("out", [TOK_PER_CORE, M], f32, kind="ExternalOutput")

    with tile.TileContext(nc) as tc:
        with (
            tc.tile_pool(name="wg", bufs=prefetch + 3) as wpool,
            tc.tile_pool(name="xt", bufs=1) as xpool,
            tc.tile_pool(name="idxt", bufs=1) as ipool,
            tc.tile_pool(name="bt", bufs=1) as bpool,
            tc.tile_pool(name="prod", bufs=pbufs) as ppool,
            tc.tile_pool(name="rest", bufs=4) as rpool,
            tc.tile_pool(name="relut", bufs=4) as relupool,
            tc.tile_pool(name="dumpa", bufs=2) as dapool,
        ):
            # preload indices (512 KB), the x shard (1 MB fp16) and the
            # device-order biases (128 KB) upfront: keeps the steady-state
            # DMA stream pure gather.
            it_all = ipool.tile([P, GROUPS * NCHUNK * (NIDX // 16)], i16)
            nc.sync.dma_start(it_all[:], idx_d[:, :])
            x_all = xpool.tile([P, GROUPS * HIDDEN], f16)
            nc.sync.dma_start(x_all[:], x_d[:, :])
            b_all = bpool.tile([P, GROUPS * M], f32)
            nc.sync.dma_start(b_all[:], b_d[:, :])

            # chunk schedule: (g, h, split?) — the very last chunk of a
            # repeats=1 build is executed as two 512-row gathers to halve
            # the end-of-kernel tail.
            chunks = [
                (g, h)
                for _ in range(repeats)
                for g in range(GROUPS)
                for h in range(NCHUNK)
            ]
            K = len(chunks)

            def issue_gather(ci):
                g, h = chunks[ci]
                k = g * NCHUNK + h
                it = it_all[:, k * (NIDX // 16) : (k + 1) * (NIDX // 16)]
                split = ci == K - 1 and repeats == 1
                if split:
                    tiles = []
                    for sub in range(2):
                        it_s = it[:, sub * 32 : (sub + 1) * 32]
                        w_t = wpool.tile([P, (M_TILE // 2) * ROW], f16)
                        nc.gpsimd.dma_gather(
                            out_ap=w_t[:].rearrange("p (c e) -> p c e", e=ROW),
                            in_ap=w_d[BASE:, :],
                            idxs_ap=it_s,
                            num_idxs=NIDX // 2,
                            num_idxs_reg=NIDX // 2,
                            elem_size=ROW,
                            queue_num=(2 * ci + sub) % queues,
                        )
                        tiles.append(w_t)
                    return tiles
                w_t = wpool.tile([P, M_TILE * ROW], f16)
                nc.gpsimd.dma_gather(
                    out_ap=w_t[:].rearrange("p (c e) -> p c e", e=ROW),
                    in_ap=w_d[BASE:, :],
                    idxs_ap=it,
                    num_idxs=NIDX,
                    num_idxs_reg=NIDX,
                    elem_size=ROW,
                    queue_num=ci % queues,
                )
                return [w_t]

            def compute_chunk(w_tiles, g, h, x_t, res_t):
                # candidate c of chunk h lives at w column block c, result
                # column mm = h*M_TILE + c.  All 8 go through the batched
                # multiply; the first na reduce on ACT, the rest on DVE.
                if len(w_tiles) == 1:
                    w_views = [(w_tiles[0], 0, M_TILE)]  # (tile, c0, ncand)
                else:
                    w_views = [
                        (w_tiles[0], 0, M_TILE // 2),
                        (w_tiles[1], M_TILE // 2, M_TILE // 2),
                    ]

                # one batched multiply per gathered view (all 8 candidates)
                prods = []  # (prod_tile, mm0, ncand)
                for w_t, c0, ncand in w_views:
                    prod = ppool.tile([P, ncand * HIDDEN], f16)
                    nc.vector.tensor_tensor(
                        out=prod[:].rearrange("p (c e) -> p c e", e=HIDDEN),
                        in0=w_t[:].rearrange("p (c e) -> p c e", e=ROW),
                        in1=x_t[:, None, :].to_broadcast([P, ncand, HIDDEN]),
                        op=mybir.AluOpType.mult,
                    )
                    prods.append((prod, h * M_TILE + c0, ncand))

                # reduces over the product slices: ACT first (it lags the
                # DVE multiply anyway), DVE takes the rest
                seq = 0  # 0..M_TILE-1 across the chunk's candidates
                for prod, mm0, n in prods:
                    for j in range(n):
                        mm = mm0 + j
                        pslice = prod[:, j * HIDDEN : (j + 1) * HIDDEN]
                        if seq < na:
                            dump = dapool.tile(
                                [P, HIDDEN],
                                mybir.dt.float8e4 if dump8 else f16,
                            )
                            nc.scalar.activation(
                                out=dump[:],
                                in_=pslice,
                                func=mybir.ActivationFunctionType.Copy,
                                accum_out=res_t[:, mm : mm + 1],
                            )
                        else:
                            nc.vector.tensor_reduce(
                                out=res_t[:, mm : mm + 1],
                                in_=pslice,
                                axis=mybir.AxisListType.X,
                                op=mybir.AluOpType.add,
                            )
                        seq += 1

                # biases for the whole chunk in one small DVE add
                cols = slice(h * M_TILE, (h + 1) * M_TILE)
                gcols = slice(g * M + h * M_TILE, g * M + (h + 1) * M_TILE)
                nc.vector.tensor_tensor(
                    out=res_t[:, cols],
                    in0=res_t[:, cols],
                    in1=b_all[:, gcols],
                    op=mybir.AluOpType.add,
                )

            pending = {}
            for ci in range(min(prefetch, K)):
                pending[ci] = issue_gather(ci)

            res_t = None
            for ci in range(K):
                g, h = chunks[ci]
                if ci + prefetch < K:
                    pending[ci + prefetch] = issue_gather(ci + prefetch)
                w_tiles = pending.pop(ci)

                if not compute:  # debug: pure gather stream timing
                    continue
                if h == 0:
                    res_t = rpool.tile([P, M], f32)
                x_t = x_all[:, g * HIDDEN : (g + 1) * HIDDEN]
                compute_chunk(w_tiles, g, h, x_t, res_t)

                if h == NCHUNK - 1:
                    relu_t = relupool.tile([P, M], f32)
                    nc.scalar.activation(
                        relu_t[:], res_t[:], mybir.ActivationFunctionType.Relu
                    )
                    nc.sync.dma_start(
                        out_d[g * P : (g + 1) * P, :], relu_t[:]
                    )

    nc.compile()
    return nc


def _get_program():
    global _cached
    if _cached is None:
        _cached = _build_program()
    return _cached


def _plan_core(idx):
    """Plan one core's gather lists.

    idx: [TOK_PER_CORE, M] int64/int32 original indices.

    Returns (packed, cand_order, drops):
      packed: [GROUPS*NCHUNK, P, NIDX//16] int16 device index input
      cand_order: [TOK_PER_CORE, M] int; device res column k of token t holds
        candidate cand_order[t, k]
      drops: list of (t, k) device res slots that the HW will drop
        (trailing-negative padding rule) and the host must fix up
    """
    idx = idx.astype(np.int64)
    idx16 = (idx - BASE).astype(np.int16)  # [T, M]

    cand_order = np.tile(np.arange(M), (TOK_PER_CORE, 1))
    # For each group, permute the partition-127 token's candidates so each
    # chunk's final gather-list slot (token 127, block M_TILE-1) is >= 0.
    for g in range(GROUPS):
        t = g * P + (P - 1)
        high = np.flatnonzero(idx[t] >= BASE)
        low = np.flatnonzero(idx[t] < BASE)
        order = np.empty(M, np.int64)
        tail_slots = [h * M_TILE + (M_TILE - 1) for h in range(NCHUNK)]
        if g == GROUPS - 1:
            # the final chunk executes as two 512-row gathers; its first
            # half's tail slot (col M - M_TILE//2 - 1) needs a high too
            tail_slots.append(M - M_TILE // 2 - 1)
        nh = min(len(high), len(tail_slots))
        order[tail_slots[:nh]] = high[:nh]
        rest = np.concatenate([high[nh:], low])
        other_slots = [k for k in range(M) if k not in tail_slots[:nh]]
        order[other_slots] = rest
        cand_order[t] = order

    eff = np.take_along_axis(idx16, cand_order, axis=1)  # [T, M] device order

    packed = np.empty((GROUPS * NCHUNK, P, NIDX // 16), np.int16)
    drops = []
    for g in range(GROUPS):
        blk = eff[g * P : (g + 1) * P]  # [128, M]
        for h in range(NCHUNK):
            lst = blk[:, h * M_TILE : (h + 1) * M_TILE].T.reshape(NIDX).copy()
            # the device executes the final chunk as two 512-row gathers;
            # apply the force-tail + trailing-drop rules per executed list
            split = g == GROUPS - 1 and h == NCHUNK - 1
            halves = [(0, NIDX // 2), (NIDX // 2, NIDX)] if split else [(0, NIDX)]
            for lo, hi in halves:
                if lst[hi - 1] < 0:
                    # A list that ends sign-negative loses its tail (and an
                    # all-negative list hard-faults the Q7) — force a valid
                    # dummy index and let the host recompute that one slot.
                    lst[hi - 1] = 0
                    drops.append(
                        (g * P + (P - 1), h * M_TILE + (hi - 1) // P)
                    )
                sub = lst[lo:hi]
                nonneg = np.flatnonzero(sub >= 0)
                last = nonneg[-1] if len(nonneg) else -1
                for i in range(last + 1, hi - lo):
                    p, c = i % P, (lo + i) // P
                    drops.append((g * P + p, h * M_TILE + c))
            wrapped = lst.reshape(NIDX // 16, 16).T  # [16, NIDX//16]
            packed[g * NCHUNK + h] = np.tile(wrapped, (8, 1))
    return packed, cand_order, drops


def _build_in_maps(input_value, mask_tensor, weight, bias):
    """Device input maps (one per core) + per-core plans for unpacking."""
    x = np.ascontiguousarray(
        np.asarray(input_value).reshape(TOKENS, HIDDEN), dtype=np.float32
    )
    idx = np.asarray(mask_tensor).reshape(TOKENS, M)

    w16 = np.ascontiguousarray(np.asarray(weight, np.float32).astype(np.float16))
    b32 = np.asarray(bias, np.float32)

    in_maps = []
    plans = []
    for c in range(N_CORES):
        t = slice(c * TOK_PER_CORE, (c + 1) * TOK_PER_CORE)
        packed, cand_order, drops = _plan_core(idx[t])
        plans.append((cand_order, drops))
        # device layouts: idx [P, k*s] (partition-major), x [P, g*HIDDEN]
        packed_pm = np.ascontiguousarray(
            packed.transpose(1, 0, 2).reshape(P, -1)
        )
        x_pm = np.ascontiguousarray(
            x[t]
            .reshape(GROUPS, P, HIDDEN)
            .transpose(1, 0, 2)
            .reshape(P, -1)
            .astype(np.float16)
        )
        # device-order biases: b_pm[p, g*M + m] = bias[idx[g*P+p, order[m]]]
        bsel = np.take_along_axis(b32[idx[t]], cand_order, axis=1)
        b_pm = np.ascontiguousarray(
            bsel.reshape(GROUPS, P, M).transpose(1, 0, 2).reshape(P, -1)
        ).astype(np.float32)
        in_maps.append({"x": x_pm, "idx": packed_pm, "w": w16, "b": b_pm})
    return in_maps, plans


def kernel(input_value, mask_tensor, weight, bias):
    from concourse.bass_utils import run_bass_kernel_spmd

    x = np.ascontiguousarray(
        np.asarray(input_value).reshape(TOKENS, HIDDEN), dtype=np.float32
    )
    idx = np.asarray(mask_tensor).reshape(TOKENS, M)

    nc = _get_program()
    in_maps, plans = _build_in_maps(input_value, mask_tensor, weight, bias)

    res = run_bass_kernel_spmd(nc, in_maps, core_ids=list(range(N_CORES)))
    kernel._last_results = res

    outs = []
    w32 = np.asarray(weight, np.float32)
    b32 = np.asarray(bias, np.float32)
    for c in range(N_CORES):
        dev = np.array(res.results[c]["out"])  # [T, M] in device cand order
        cand_order, drops = plans[c]
        t0 = c * TOK_PER_CORE
        for t_loc, k in drops:  # exact host fixup for HW-dropped tail slots
            cand = cand_order[t_loc, k]
            v = int(idx[t0 + t_loc, cand])
            dev[t_loc, k] = max(
                float(np.dot(x[t0 + t_loc], w32[v]) + b32[v]), 0.0
            )
        out = np.empty_like(dev)
        np.put_along_axis(out, cand_order, dev, axis=1)
        outs.append(out)

    out = np.concatenate(outs, axis=0)
    return out.reshape(mask_tensor.shape).astype(np.float32)
